# revision 53
# baseline (speedup 1.0000x reference)
"""Trainium2 Bass kernel for NewPatchLoss.

Computes: mean over (N, C) of max over the 16x16-patch grid of per-patch mean
|output - target|, for output/target of shape [16, 3, 512, 512] f32.

Sharding: pure data parallel over the batch axis — each of the 8 cores gets
2 samples (= 6 [512, 512] images). The device reduces each image to per-patch
(or per-(patch-row, column)) partial sums; the host combines the tiny
partials (final 16-col sums for the last images, max over patches, /256,
clamp at 0, mean over 48).

The problem is memory-bound: inputs stream as bf16 (host converts; the 0.4%
quantization is far inside the 2e-2 tolerance), 6.3 MB per core at ~330-390
GB/s on the sync-engine HWDGE ring. Unit sizes are graduated so compute
starts early and the post-stream drain is short:
  img0: 2 x 512 KB units, imgs 1-3: 1 MB units, img4: 2 x 512 KB,
  img5: 4 x 256 KB units, streamed in that order.

Per-unit pipeline:
  1. DMA unit (cols [x | y] interleaved so one transfer feeds one subtract).
  2. DVE tensor_tensor SUBTRACT (bf16, 2x mode): d = x - y.
  3. e = |d|: ScalarE ACTIVATE Abs for imgs 1-3 (latency hides behind the
     stream), DVE bitwise_and 0x7FFF7FFF on the u32 view (sign-bit clear)
     for the units near the end of the stream (img0 shares the DVE early).
  4. PE: per 512-col row-chunk c, a matmul with a 0/1 block matrix
     lhsT[128, 32] sums 16 image rows -> per-(patch-row, column) sums.
     Imgs 0-3 accumulate 4 chunks into a 32-partition PSUM slice
     (ps01/ps23); imgs 4, 5 use independent per-chunk matmuls scattered
     into four [64, 512] tiles sc[k] (img4 chunk k on partitions 0:32,
     img5 chunk k on 32:64) so no accumulation chain serializes the tail.
  5. Evacuation: DVE segmented reduce [64, (32,16)] -> 32 patch sums per
     partition for ps01, ps23 (mid-stream, free) and sc[3] (tail) into one
     grid tile; sc[0..2] are copied PSUM->SBUF by the otherwise-idle
     ScalarE after the stream ends and shipped raw (128 KB each) — the
     host finishes their 16-col sums, avoiding a pile of in-order DVE
     reduces behind the last-landing units.

Engine notes baked into this structure (measured on hardware):
  - exec time ~= end of the last result DMA; the framework pre/postamble
    (~8.7 us to first DMA byte) is fixed.
  - The DVE is the scarce engine; its in-order queue must never contain an
    op that waits on a late matmul (emission order = execution order).
  - GpSimd elementwise ops contend with the DVE for the shared SBUF port
    (DVE tensor ops slowed ~2x while GpSimd ran) — GpSimd is unused.
  - The scalar-engine HWDGE ring is ~2x slower than the sync ring and
    shares the 16 SDMA engines, so all bulk DMAs go on the sync ring only
    (the tiny ones_blk load uses the scalar ring).

BASSK_TRACE=1 captures an NTFF profile and fills LAST_RESULTS.exec_time_ns.
"""

import os
import numpy as np
from contextlib import ExitStack

N, C, H, W = 16, 3, 512, 512
P = 16  # patch size
N_CORES = 8
IMGS = (N // N_CORES) * C  # images per core = 6

_cache = {}
LAST_RESULTS = None  # BassKernelResults of the most recent run (for test.py)
LAST_TRACE_DIR = None


def _install_ntff_hook():
    """Provide antenv.axon_hooks.get_axon_ntff_profile_hook via ctypes on
    libaxon_pjrt.so when the real antenv package isn't shipped (used only
    for profiling runs, BASSK_TRACE=1)."""
    import sys
    import types
    import contextlib
    import ctypes

    try:
        from antenv.axon_hooks import get_axon_ntff_profile_hook  # noqa: F401

        return
    except ImportError:
        pass

    hook = None
    try:
        lib = ctypes.CDLL("/opt/axon/libaxon_pjrt.so")
        if hasattr(lib, "axon_start_nrt_profile"):
            lib.axon_start_nrt_profile.argtypes = [
                ctypes.POINTER(ctypes.c_int64),
                ctypes.c_size_t,
            ]
            lib.axon_start_nrt_profile.restype = ctypes.c_int64
            lib.axon_stop_nrt_profile.argtypes = [ctypes.c_char_p]
            lib.axon_stop_nrt_profile.restype = ctypes.c_int64

            @contextlib.contextmanager
            def _hook(output_dir, device_ids):
                import jax

                jax.devices()
                if device_ids:
                    ids = (ctypes.c_int64 * len(device_ids))(*device_ids)
                    rc = lib.axon_start_nrt_profile(ids, len(device_ids))
                else:
                    rc = lib.axon_start_nrt_profile(None, 0)
                if rc != 0:
                    raise RuntimeError(f"axon_start_nrt_profile rc={rc}")
                try:
                    yield
                finally:
                    n = lib.axon_stop_nrt_profile(str(output_dir).encode())
                    print(f"ntff profile: {n} file(s) -> {output_dir}")

            hook = _hook
    except OSError:
        hook = None

    mod = types.ModuleType("antenv.axon_hooks")
    mod.get_axon_ntff_profile_hook = lambda: hook
    sys.modules["antenv.axon_hooks"] = mod


def _numpy_fallback(output, target):
    """Host-side computation, used only if the device path fails twice."""
    o = np.asarray(output, np.float32)
    t = np.asarray(target, np.float32)
    d = np.abs(o - t)
    pl = d.reshape(N, C, H // P, P, W // P, P).mean(axis=(3, 5), dtype=np.float32)
    mx = np.maximum(pl.max(axis=(2, 3)), np.float32(0.0))
    return np.float32(mx.mean(dtype=np.float32))


def _build():
    import concourse.tile as tile
    from concourse import bacc, mybir

    f32 = mybir.dt.float32
    bf16 = mybir.dt.bfloat16
    u32 = mybir.dt.uint32
    nc = bacc.Bacc("TRN2", debug=False, enable_asserts=False, num_devices=N_CORES)
    # img0 as two 512 KB units (fast pipeline fill): unit h = image rows
    # [256h, 256h+256) as two 512-col row-chunks; cols 0:1024 x, 1024:2048 y.
    xb0 = nc.dram_tensor("xy_img0", [2, 128, 2048], bf16, kind="ExternalInput").ap()
    # imgs 1..3 as 512 KB fp8 (e3m4) units: cols 0:2048 x, 2048:4096 y.
    # Their DVE subtract runs 1x (2.3 us/img) but fits the mid-stream
    # slack, while the stream sheds 1.5 MB; abs stays on ScalarE.
    f8 = mybir.dt.float8e3
    xb = nc.dram_tensor("xy_big", [2, 128, 4096], f8, kind="ExternalInput").ap()
    # img3 stays bf16: the DVE (1x fp8 subtract) is the bottleneck by the
    # time img3 lands, so trading 0.5 MB of stream for a 2x-mode subtract wins
    xb3 = nc.dram_tensor("xy_img3", [128, 4096], bf16, kind="ExternalInput").ap()
    # img4 as two 512 KB units (spread the late DVE work)
    xb4 = nc.dram_tensor("xy_img4", [2, 128, 2048], bf16, kind="ExternalInput").ap()
    # img5 as four 256 KB units (short tail): row-chunk s; 0:512 x, 512:1024 y.
    xs = nc.dram_tensor("xy_small", [4, 128, 1024], bf16, kind="ExternalInput").ap()
    ones = nc.dram_tensor("ones_blk", [128, 128], bf16, kind="ExternalInput").ap()
    res = nc.dram_tensor("res", [64, 32], f32, kind="ExternalOutput").ap()
    # raw per-(patch-row, column) 16-row sums for imgs 4,5: the host does the
    # final 16-col sums + maxes for these (ScalarE evacuates PSUM post-stream)
    res_sc = nc.dram_tensor("res_sc", [5, 64, 512], f32, kind="ExternalOutput").ap()

    MASK = 0x7FFF7FFF  # clears the sign bit of two packed bf16

    with tile.TileContext(nc) as tc, ExitStack() as ctx:
        pool_i0 = ctx.enter_context(tc.tile_pool(name="inp0", bufs=2))
        pool_in = ctx.enter_context(tc.tile_pool(name="inp", bufs=4))
        pool_ins = ctx.enter_context(tc.tile_pool(name="inps", bufs=4))
        pool_d = ctx.enter_context(tc.tile_pool(name="dif", bufs=6))
        pool_g = ctx.enter_context(tc.tile_pool(name="grid", bufs=3))
        pool_ps = ctx.enter_context(tc.tile_pool(name="ps", bufs=1, space="PSUM"))
        pool_misc = ctx.enter_context(tc.tile_pool(name="misc", bufs=1))
        pool_cp = ctx.enter_context(tc.tile_pool(name="cp", bufs=4))

        # stream order: img0 halves, then imgs 1-2 (fp8), img3, img4
        # halves, img5 quarters — all on the sync HWDGE ring (the scalar
        # ring measured ~2x slower, and sharing SDMA engines across rings
        # delays the first unit's landing)
        t_i0 = []
        for h in range(2):
            t = pool_i0.tile([128, 2048], bf16, tag="xy0")
            nc.sync.dma_start(t[:], xb0[h, :, :])
            t_i0.append(t)
            if h == 0:
                # scalar ring: lands without delaying the main stream
                onesb = pool_misc.tile([128, 128], bf16)
                nc.scalar.dma_start(onesb[:], ones)
                grid = pool_misc.tile([64, 32], f32)
        t_big = []
        for u in range(2):
            t = pool_in.tile([128, 4096], f8, tag="xyb")
            nc.sync.dma_start(t[:], xb[u, :, :])
            t_big.append(t)
        t3 = pool_in.tile([128, 4096], bf16, tag="xyb3")
        nc.sync.dma_start(t3[:], xb3)
        t_big.append(t3)
        t_i4 = []
        for h in range(2):
            t = pool_i0.tile([128, 2048], bf16, tag="xy4")
            nc.sync.dma_start(t[:], xb4[h, :, :])
            t_i4.append(t)
        t_small = []
        for s in range(4):
            t = pool_ins.tile([128, 1024], bf16, tag="xys")
            nc.sync.dma_start(t[:], xs[s, :, :])
            t_small.append(t)

        # ps01: imgs 0,1 / ps23: imgs 2,3 (accumulated, 32-part slices)
        # s[k]: chunk k of img4 -> [0:32] and of img5 -> [32:64], each an
        # independent matmul (block-0 lhsT), so reduces fire incrementally
        ps01 = pool_ps.tile([64, 512], f32, tag="ps01", name="ps01")
        ps23 = pool_ps.tile([64, 512], f32, tag="ps23", name="ps23")
        sc = [
            pool_ps.tile([64, 512], f32, tag=f"sc{k}", name=f"sc{k}")
            for k in range(4)
        ]

        def mm_acc(e_ap, i, c):
            # chunk c holds patch-rows 8c..8c+7 of image i (0..3)
            out = ps01 if i < 2 else ps23
            lo = 32 * (i % 2)
            nc.tensor.matmul(
                out[lo : lo + 32, :],
                onesb[:, 32 * c : 32 * c + 32],
                e_ap,
                start=(c == 0),
                stop=(c == 3),
            )

        def mm_scatter(e_ap, k, half):
            # img4 (half 0) / img5 (half 1), chunk k: independent matmul
            nc.tensor.matmul(
                sc[k][32 * half : 32 * half + 32, :],
                onesb[:, 0:32],
                e_ap,
                start=True,
                stop=True,
            )

        def reduce_ps(src_ap, n_part, col):
            # PSUM -> 16-col patch sums; the tiny grid goes to the host,
            # which finishes max/divide/mean (no max-reduce in the tail)
            nc.vector.tensor_reduce(
                grid[0:n_part, 32 * col : 32 * col + 32],
                src_ap.rearrange("p (c w) -> p c w", w=P),
                axis=mybir.AxisListType.X,
                op=mybir.AluOpType.add,
            )

        def sub_abs(t, w, on_scalar):
            # t[:, 0:w] - t[:, w:2w] -> |.| -> e
            d = pool_d.tile([128, 2048], bf16, tag="d")
            nc.vector.tensor_tensor(
                d[:, 0:w], t[:, 0:w], t[:, w : 2 * w], op=mybir.AluOpType.subtract
            )
            e = pool_d.tile([128, 2048], bf16, tag="e")
            if on_scalar:
                nc.scalar.activation(
                    e[:, 0:w], d[:, 0:w], mybir.ActivationFunctionType.Abs
                )
            else:
                nc.vector.tensor_scalar(
                    e[:, 0:w].bitcast(u32),
                    d[:, 0:w].bitcast(u32),
                    MASK,
                    None,
                    op0=mybir.AluOpType.bitwise_and,
                )
            return e

        for h in range(2):
            e = sub_abs(t_i0[h], 1024, on_scalar=True)
            mm_acc(e[:, 0:512], 0, 2 * h)
            mm_acc(e[:, 512:1024], 0, 2 * h + 1)

        for u in range(3):
            i = u + 1
            e = sub_abs(t_big[u], 2048, on_scalar=True)
            for j in range(4):
                mm_acc(e[:, 512 * j : 512 * j + 512], i, j)

        for h in range(2):
            e = sub_abs(t_i4[h], 1024, on_scalar=False)
            mm_scatter(e[:, 0:512], 2 * h, 0)
            mm_scatter(e[:, 512:1024], 2 * h + 1, 0)

        # small units: keep the in-order DVE queue free of anything that
        # waits on late matmuls; chunk pairs 0-2 are evacuated raw via the
        # otherwise-idle ScalarE (the host finishes their 16-col sums), and
        # only chunk pair 3 takes a DVE reduce straight into the grid
        def ship_raw(src_tile, slot):
            # evacuate PSUM via the otherwise-idle ScalarE and ship it raw;
            # the host finishes the 16-col patch sums
            cp = pool_cp.tile([64, 512], f32, tag="cp", name="cp")
            nc.scalar.copy(cp[:], src_tile[:])
            nc.sync.dma_start(res_sc[slot, :, :], cp[:])

        for s in range(4):
            e = sub_abs(t_small[s], 512, on_scalar=False)
            mm_scatter(e[:, 0:512], s, 1)
            if s == 0:
                ship_raw(ps01, 0)
                ship_raw(ps23, 1)
            if s > 0:
                ship_raw(sc[s - 1], 1 + s)

        reduce_ps(sc[3][:], 64, 0)
        nc.sync.dma_start(res, grid[:])

    nc.compile()
    return nc


def _ones_blk():
    import ml_dtypes

    # column group c (32 cols): col m hot for partitions 16(m-8c)..+16
    o = np.zeros((128, 128), np.float32)
    p = np.arange(128)
    for c in range(4):
        o[p, 32 * c + 8 * c + p // 16] = 1.0
    return o.astype(ml_dtypes.bfloat16)


def _pack(output, target):
    import ml_dtypes

    bf = ml_dtypes.bfloat16
    # [core, img, h, sub, p, w]: image row = 256h + 128 sub + p
    x = np.asarray(output).reshape(N_CORES, IMGS, 2, 2, 128, 512).astype(bf)
    y = np.asarray(target).reshape(N_CORES, IMGS, 2, 2, 128, 512).astype(bf)

    def blocks(a):
        # [core, block, p, (sub, w)] where block = 2*img + h
        return np.ascontiguousarray(a.transpose(0, 1, 2, 4, 3, 5)).reshape(
            N_CORES, 2 * IMGS, 128, 1024
        )

    bx, by = blocks(x), blocks(y)
    # img0 units: blocks 0,1 -> [core, 2, p, 2048] (x block | y block)
    xy_img0 = np.ascontiguousarray(np.concatenate([bx[:, :2], by[:, :2]], axis=3))
    # full-image units for imgs 1..3: block pairs (2i, 2i+1)
    bigx = bx[:, 2:8].transpose(0, 2, 1, 3).reshape(N_CORES, 128, 3, 2048)
    bigy = by[:, 2:8].transpose(0, 2, 1, 3).reshape(N_CORES, 128, 3, 2048)
    big = np.concatenate([bigx, bigy], axis=3).transpose(0, 2, 1, 3)
    xy_big = np.ascontiguousarray(big[:, 0:2]).astype(ml_dtypes.float8_e3m4)
    xy_img3 = np.ascontiguousarray(big[:, 2])  # [core, 128, 4096] bf16
    # img4 units: blocks 8,9 -> [core, 2, p, 2048]
    xy_img4 = np.ascontiguousarray(np.concatenate([bx[:, 8:10], by[:, 8:10]], axis=3))
    # small units: image 5 row-chunks c = 2h + sub -> [core, 4, p, 512]
    sx = x[:, 5].reshape(N_CORES, 4, 128, 512)
    sy = y[:, 5].reshape(N_CORES, 4, 128, 512)
    xy_small = np.ascontiguousarray(np.concatenate([sx, sy], axis=3))
    return xy_img0, xy_big, xy_img3, xy_img4, xy_small


def kernel(output, target, patch_size):
    global LAST_RESULTS
    assert int(patch_size) == P
    try:
        return _kernel_device(output, target)
    except Exception:
        import time
        import traceback

        traceback.print_exc()
        time.sleep(3)
        try:
            return _kernel_device(output, target)
        except Exception:
            traceback.print_exc()
            return _numpy_fallback(output, target)


def _kernel_device(output, target):
    global LAST_RESULTS
    from concourse import bass_utils
    from concourse.bass_interp import get_hw_module

    if "nc" not in _cache:
        _cache["nc"] = _build()
    nc = _cache["nc"]

    xy_img0, xy_big, xy_img3, xy_img4, xy_small = _pack(output, target)
    ones = _ones_blk()
    in_maps = [
        {
            "xy_img0": xy_img0[i],
            "xy_big": xy_big[i],
            "xy_img3": xy_img3[i],
            "xy_img4": xy_img4[i],
            "xy_small": xy_small[i],
            "ones_blk": ones,
        }
        for i in range(N_CORES)
    ]

    trace = bool(int(os.environ.get("BASSK_TRACE", "0")))
    tmpdir = None
    if trace:
        import tempfile

        _install_ntff_hook()
        tmpdir = tempfile.mkdtemp(prefix="bassk_trace_")
        global LAST_TRACE_DIR
        LAST_TRACE_DIR = tmpdir
    old_m = nc.m
    nc.m = get_hw_module(nc.m)
    try:
        results = bass_utils.run_bass_kernel_spmd(
            nc, in_maps, core_ids=list(range(N_CORES)), trace=trace, tmpdir=tmpdir
        )
    finally:
        nc.m = old_m
    LAST_RESULTS = results

    vals = np.stack([r["res"] for r in results.results])  # [8, 64, 32]
    scv = np.stack([r["res_sc"] for r in results.results])  # [8, 5, 64, 512]
    # res = patch-sum grid for imgs 4,5 chunk 3; res_sc slots: ps01, ps23,
    # sc0, sc1, sc2 (rows 0:32 even image / 32:64 odd image of each pair).
    # Finish the 16-col patch sums for the raw tiles on the host.
    s = scv.reshape(N_CORES, 5, 64, 32, 16).sum(axis=4, dtype=np.float32)
    i4 = np.maximum(
        s[:, 2:5, 0:32].max(axis=(1, 2, 3)), vals[:, 0:32].max(axis=(1, 2))
    )
    i5 = np.maximum(
        s[:, 2:5, 32:64].max(axis=(1, 2, 3)), vals[:, 32:64].max(axis=(1, 2))
    )
    mx = np.stack(
        [
            s[:, 0, 0:32].max(axis=(1, 2)),
            s[:, 0, 32:64].max(axis=(1, 2)),
            s[:, 1, 0:32].max(axis=(1, 2)),
            s[:, 1, 32:64].max(axis=(1, 2)),
            i4,
            i5,
        ],
        axis=1,
    )  # [8, 6]
    max_patch_loss = np.maximum(mx.astype(np.float32) / np.float32(P * P), 0.0)
    return np.float32(max_patch_loss.mean(dtype=np.float32))


# revision 54
# speedup vs baseline: 1.0553x; 1.0553x over previous
"""Trainium2 Bass kernel for NewPatchLoss.

Computes: mean over (N, C) of max over the 16x16-patch grid of per-patch mean
|output - target|, for output/target of shape [16, 3, 512, 512] f32.

Sharding: pure data parallel over the batch axis — each of the 8 cores gets
2 samples (= 6 [512, 512] images). The device reduces each image to per-patch
(or per-(patch-row, column)) partial sums; the host combines the tiny
partials (final 16-col sums for the last images, max over patches, /256,
clamp at 0, mean over 48).

The problem is memory-bound: inputs stream as bf16 (host converts; the 0.4%
quantization is far inside the 2e-2 tolerance), 6.3 MB per core at ~330-390
GB/s on the sync-engine HWDGE ring. Unit sizes are graduated so compute
starts early and the post-stream drain is short:
  img0: 2 x 512 KB units, imgs 1-3: 1 MB units, img4: 2 x 512 KB,
  img5: 4 x 256 KB units, streamed in that order.

Per-unit pipeline:
  1. DMA unit (cols [x | y] interleaved so one transfer feeds one subtract).
  2. DVE tensor_tensor SUBTRACT (bf16, 2x mode): d = x - y.
  3. e = |d|: ScalarE ACTIVATE Abs for imgs 1-3 (latency hides behind the
     stream), DVE bitwise_and 0x7FFF7FFF on the u32 view (sign-bit clear)
     for the units near the end of the stream (img0 shares the DVE early).
  4. PE: per 512-col row-chunk c, a matmul with a 0/1 block matrix
     lhsT[128, 32] sums 16 image rows -> per-(patch-row, column) sums.
     Imgs 0-3 accumulate 4 chunks into a 32-partition PSUM slice
     (ps01/ps23); imgs 4, 5 use independent per-chunk matmuls scattered
     into four [64, 512] tiles sc[k] (img4 chunk k on partitions 0:32,
     img5 chunk k on 32:64) so no accumulation chain serializes the tail.
  5. Evacuation: DVE segmented reduce [64, (32,16)] -> 32 patch sums per
     partition for ps01, ps23 (mid-stream, free) and sc[3] (tail) into one
     grid tile; sc[0..2] are copied PSUM->SBUF by the otherwise-idle
     ScalarE after the stream ends and shipped raw (128 KB each) — the
     host finishes their 16-col sums, avoiding a pile of in-order DVE
     reduces behind the last-landing units.

Engine notes baked into this structure (measured on hardware):
  - exec time ~= end of the last result DMA; the framework pre/postamble
    (~8.7 us to first DMA byte) is fixed.
  - The DVE is the scarce engine; its in-order queue must never contain an
    op that waits on a late matmul (emission order = execution order).
  - GpSimd elementwise ops contend with the DVE for the shared SBUF port
    (DVE tensor ops slowed ~2x while GpSimd ran) — GpSimd is unused.
  - The scalar-engine HWDGE ring is ~2x slower than the sync ring and
    shares the 16 SDMA engines, so all bulk DMAs go on the sync ring only
    (the tiny ones_blk load uses the scalar ring).

BASSK_TRACE=1 captures an NTFF profile and fills LAST_RESULTS.exec_time_ns.
"""

import os
import numpy as np
from contextlib import ExitStack

N, C, H, W = 16, 3, 512, 512
P = 16  # patch size
N_CORES = 8
IMGS = (N // N_CORES) * C  # images per core = 6

_cache = {}
LAST_RESULTS = None  # BassKernelResults of the most recent run (for test.py)
LAST_TRACE_DIR = None


def _install_ntff_hook():
    """Provide antenv.axon_hooks.get_axon_ntff_profile_hook via ctypes on
    libaxon_pjrt.so when the real antenv package isn't shipped (used only
    for profiling runs, BASSK_TRACE=1)."""
    import sys
    import types
    import contextlib
    import ctypes

    try:
        from antenv.axon_hooks import get_axon_ntff_profile_hook  # noqa: F401

        return
    except ImportError:
        pass

    hook = None
    try:
        lib = ctypes.CDLL("/opt/axon/libaxon_pjrt.so")
        if hasattr(lib, "axon_start_nrt_profile"):
            lib.axon_start_nrt_profile.argtypes = [
                ctypes.POINTER(ctypes.c_int64),
                ctypes.c_size_t,
            ]
            lib.axon_start_nrt_profile.restype = ctypes.c_int64
            lib.axon_stop_nrt_profile.argtypes = [ctypes.c_char_p]
            lib.axon_stop_nrt_profile.restype = ctypes.c_int64

            @contextlib.contextmanager
            def _hook(output_dir, device_ids):
                import jax

                jax.devices()
                if device_ids:
                    ids = (ctypes.c_int64 * len(device_ids))(*device_ids)
                    rc = lib.axon_start_nrt_profile(ids, len(device_ids))
                else:
                    rc = lib.axon_start_nrt_profile(None, 0)
                if rc != 0:
                    raise RuntimeError(f"axon_start_nrt_profile rc={rc}")
                try:
                    yield
                finally:
                    n = lib.axon_stop_nrt_profile(str(output_dir).encode())
                    print(f"ntff profile: {n} file(s) -> {output_dir}")

            hook = _hook
    except OSError:
        hook = None

    mod = types.ModuleType("antenv.axon_hooks")
    mod.get_axon_ntff_profile_hook = lambda: hook
    sys.modules["antenv.axon_hooks"] = mod


def _numpy_fallback(output, target):
    """Host-side computation, used only if the device path fails twice."""
    o = np.asarray(output, np.float32)
    t = np.asarray(target, np.float32)
    d = np.abs(o - t)
    pl = d.reshape(N, C, H // P, P, W // P, P).mean(axis=(3, 5), dtype=np.float32)
    mx = np.maximum(pl.max(axis=(2, 3)), np.float32(0.0))
    return np.float32(mx.mean(dtype=np.float32))


def _build():
    import concourse.tile as tile
    from concourse import bacc, mybir

    f32 = mybir.dt.float32
    f8 = mybir.dt.float8e3
    bf16 = mybir.dt.bfloat16
    u32 = mybir.dt.uint32
    nc = bacc.Bacc("TRN2", debug=False, enable_asserts=False, num_devices=N_CORES)
    # img0 as two 256 KB fp8 units (fast pipeline fill + early DVE work):
    # unit h = image rows [256h, 256h+256); cols 0:1024 x, 1024:2048 y.
    xb0 = nc.dram_tensor("xy_img0", [2, 128, 2048], f8, kind="ExternalInput").ap()
    # imgs 1..3 as 512 KB fp8 (e3m4) units: cols 0:2048 x, 2048:4096 y.
    # Their DVE subtract runs 1x (2.3 us/img) but fits the mid-stream
    # slack, while the stream sheds 1.5 MB; abs stays on ScalarE.
    xb = nc.dram_tensor("xy_big", [2, 128, 4096], f8, kind="ExternalInput").ap()
    # img3 stays bf16: the DVE (1x fp8 subtract) is the bottleneck by the
    # time img3 lands, so trading 0.5 MB of stream for a 2x-mode subtract wins
    xb3 = nc.dram_tensor("xy_img3", [128, 4096], bf16, kind="ExternalInput").ap()
    # img4 as two 512 KB units (spread the late DVE work)
    xb4 = nc.dram_tensor("xy_img4", [2, 128, 2048], bf16, kind="ExternalInput").ap()
    # img5 as four 256 KB units (short tail): row-chunk s; 0:512 x, 512:1024 y.
    xs = nc.dram_tensor("xy_small", [4, 128, 1024], bf16, kind="ExternalInput").ap()
    ones = nc.dram_tensor("ones_blk", [128, 128], bf16, kind="ExternalInput").ap()
    res = nc.dram_tensor("res", [64, 32], f32, kind="ExternalOutput").ap()
    # raw per-(patch-row, column) 16-row sums for imgs 4,5: the host does the
    # final 16-col sums + maxes for these (ScalarE evacuates PSUM post-stream)
    res_sc = nc.dram_tensor("res_sc", [5, 64, 512], f32, kind="ExternalOutput").ap()

    MASK = 0x7FFF7FFF  # clears the sign bit of two packed bf16

    with tile.TileContext(nc) as tc, ExitStack() as ctx:
        pool_i0 = ctx.enter_context(tc.tile_pool(name="inp0", bufs=2))
        pool_in = ctx.enter_context(tc.tile_pool(name="inp", bufs=4))
        pool_ins = ctx.enter_context(tc.tile_pool(name="inps", bufs=4))
        pool_d = ctx.enter_context(tc.tile_pool(name="dif", bufs=6))
        pool_g = ctx.enter_context(tc.tile_pool(name="grid", bufs=3))
        pool_ps = ctx.enter_context(tc.tile_pool(name="ps", bufs=1, space="PSUM"))
        pool_misc = ctx.enter_context(tc.tile_pool(name="misc", bufs=1))
        pool_cp = ctx.enter_context(tc.tile_pool(name="cp", bufs=4))

        # stream order: img0 halves, then imgs 1-2 (fp8), img3, img4
        # halves, img5 quarters — all on the sync HWDGE ring (the scalar
        # ring measured ~2x slower, and sharing SDMA engines across rings
        # delays the first unit's landing)
        t_i0 = []
        for h in range(2):
            t = pool_i0.tile([128, 2048], f8, tag="xy0")
            nc.sync.dma_start(t[:], xb0[h, :, :])
            t_i0.append(t)
            if h == 0:
                # scalar ring: lands without delaying the main stream
                onesb = pool_misc.tile([128, 128], bf16)
                nc.scalar.dma_start(onesb[:], ones)
                grid = pool_misc.tile([64, 32], f32)
        t_big = []
        for u in range(2):
            t = pool_in.tile([128, 4096], f8, tag="xyb")
            nc.sync.dma_start(t[:], xb[u, :, :])
            t_big.append(t)
        t3 = pool_in.tile([128, 4096], bf16, tag="xyb3")
        nc.sync.dma_start(t3[:], xb3)
        t_big.append(t3)
        t_i4 = []
        for h in range(2):
            t = pool_i0.tile([128, 2048], bf16, tag="xy4")
            nc.sync.dma_start(t[:], xb4[h, :, :])
            t_i4.append(t)
        t_small = []
        for s in range(4):
            t = pool_ins.tile([128, 1024], bf16, tag="xys")
            nc.sync.dma_start(t[:], xs[s, :, :])
            t_small.append(t)

        # ps01: imgs 0,1 / ps23: imgs 2,3 (accumulated, 32-part slices)
        # s[k]: chunk k of img4 -> [0:32] and of img5 -> [32:64], each an
        # independent matmul (block-0 lhsT), so reduces fire incrementally
        ps01 = pool_ps.tile([64, 512], f32, tag="ps01", name="ps01")
        ps23 = pool_ps.tile([64, 512], f32, tag="ps23", name="ps23")
        sc = [
            pool_ps.tile([64, 512], f32, tag=f"sc{k}", name=f"sc{k}")
            for k in range(4)
        ]

        def mm_acc(e_ap, i, c):
            # chunk c holds patch-rows 8c..8c+7 of image i (0..3)
            out = ps01 if i < 2 else ps23
            lo = 32 * (i % 2)
            nc.tensor.matmul(
                out[lo : lo + 32, :],
                onesb[:, 32 * c : 32 * c + 32],
                e_ap,
                start=(c == 0),
                stop=(c == 3),
            )

        def mm_scatter(e_ap, k, half):
            # img4 (half 0) / img5 (half 1), chunk k: independent matmul
            nc.tensor.matmul(
                sc[k][32 * half : 32 * half + 32, :],
                onesb[:, 0:32],
                e_ap,
                start=True,
                stop=True,
            )

        def reduce_ps(src_ap, n_part, col):
            # PSUM -> 16-col patch sums; the tiny grid goes to the host,
            # which finishes max/divide/mean (no max-reduce in the tail)
            nc.vector.tensor_reduce(
                grid[0:n_part, 32 * col : 32 * col + 32],
                src_ap.rearrange("p (c w) -> p c w", w=P),
                axis=mybir.AxisListType.X,
                op=mybir.AluOpType.add,
            )

        def sub_abs(t, w, on_scalar):
            # t[:, 0:w] - t[:, w:2w] -> |.| -> e
            d = pool_d.tile([128, 2048], bf16, tag="d")
            nc.vector.tensor_tensor(
                d[:, 0:w], t[:, 0:w], t[:, w : 2 * w], op=mybir.AluOpType.subtract
            )
            e = pool_d.tile([128, 2048], bf16, tag="e")
            if on_scalar:
                nc.scalar.activation(
                    e[:, 0:w], d[:, 0:w], mybir.ActivationFunctionType.Abs
                )
            else:
                nc.vector.tensor_scalar(
                    e[:, 0:w].bitcast(u32),
                    d[:, 0:w].bitcast(u32),
                    MASK,
                    None,
                    op0=mybir.AluOpType.bitwise_and,
                )
            return e

        for h in range(2):
            e = sub_abs(t_i0[h], 1024, on_scalar=True)
            mm_acc(e[:, 0:512], 0, 2 * h)
            mm_acc(e[:, 512:1024], 0, 2 * h + 1)

        for u in range(3):
            i = u + 1
            e = sub_abs(t_big[u], 2048, on_scalar=True)
            for j in range(4):
                mm_acc(e[:, 512 * j : 512 * j + 512], i, j)

        for h in range(2):
            e = sub_abs(t_i4[h], 1024, on_scalar=False)
            mm_scatter(e[:, 0:512], 2 * h, 0)
            mm_scatter(e[:, 512:1024], 2 * h + 1, 0)

        # small units: keep the in-order DVE queue free of anything that
        # waits on late matmuls; chunk pairs 0-2 are evacuated raw via the
        # otherwise-idle ScalarE (the host finishes their 16-col sums), and
        # only chunk pair 3 takes a DVE reduce straight into the grid
        def ship_raw(src_tile, slot):
            # evacuate PSUM via the otherwise-idle ScalarE and ship it raw;
            # the host finishes the 16-col patch sums
            cp = pool_cp.tile([64, 512], f32, tag="cp", name="cp")
            nc.scalar.copy(cp[:], src_tile[:])
            nc.sync.dma_start(res_sc[slot, :, :], cp[:])

        for s in range(4):
            e = sub_abs(t_small[s], 512, on_scalar=False)
            mm_scatter(e[:, 0:512], s, 1)
            if s == 0:
                ship_raw(ps01, 0)
                ship_raw(ps23, 1)
            if s > 0:
                ship_raw(sc[s - 1], 1 + s)

        reduce_ps(sc[3][:], 64, 0)
        # scalar ring: the sync FIFO is busy draining the raw-tile DMAs
        nc.scalar.dma_start(res, grid[:])

    nc.compile()
    return nc


def _ones_blk():
    import ml_dtypes

    # column group c (32 cols): col m hot for partitions 16(m-8c)..+16
    o = np.zeros((128, 128), np.float32)
    p = np.arange(128)
    for c in range(4):
        o[p, 32 * c + 8 * c + p // 16] = 1.0
    return o.astype(ml_dtypes.bfloat16)


def _pack(output, target):
    import ml_dtypes

    bf = ml_dtypes.bfloat16
    # [core, img, h, sub, p, w]: image row = 256h + 128 sub + p
    x = np.asarray(output).reshape(N_CORES, IMGS, 2, 2, 128, 512).astype(bf)
    y = np.asarray(target).reshape(N_CORES, IMGS, 2, 2, 128, 512).astype(bf)

    def blocks(a):
        # [core, block, p, (sub, w)] where block = 2*img + h
        return np.ascontiguousarray(a.transpose(0, 1, 2, 4, 3, 5)).reshape(
            N_CORES, 2 * IMGS, 128, 1024
        )

    bx, by = blocks(x), blocks(y)
    # img0 units: blocks 0,1 -> [core, 2, p, 2048] (x block | y block), fp8
    xy_img0 = np.ascontiguousarray(
        np.concatenate([bx[:, :2], by[:, :2]], axis=3)
    ).astype(ml_dtypes.float8_e3m4)
    # full-image units for imgs 1..3: block pairs (2i, 2i+1)
    bigx = bx[:, 2:8].transpose(0, 2, 1, 3).reshape(N_CORES, 128, 3, 2048)
    bigy = by[:, 2:8].transpose(0, 2, 1, 3).reshape(N_CORES, 128, 3, 2048)
    big = np.concatenate([bigx, bigy], axis=3).transpose(0, 2, 1, 3)
    xy_big = np.ascontiguousarray(big[:, 0:2]).astype(ml_dtypes.float8_e3m4)
    xy_img3 = np.ascontiguousarray(big[:, 2])  # [core, 128, 4096] bf16
    # img4 units: blocks 8,9 -> [core, 2, p, 2048]
    xy_img4 = np.ascontiguousarray(np.concatenate([bx[:, 8:10], by[:, 8:10]], axis=3))
    # small units: image 5 row-chunks c = 2h + sub -> [core, 4, p, 512]
    sx = x[:, 5].reshape(N_CORES, 4, 128, 512)
    sy = y[:, 5].reshape(N_CORES, 4, 128, 512)
    xy_small = np.ascontiguousarray(np.concatenate([sx, sy], axis=3))
    return xy_img0, xy_big, xy_img3, xy_img4, xy_small


def kernel(output, target, patch_size):
    global LAST_RESULTS
    assert int(patch_size) == P
    try:
        return _kernel_device(output, target)
    except Exception:
        import time
        import traceback

        traceback.print_exc()
        time.sleep(3)
        try:
            return _kernel_device(output, target)
        except Exception:
            traceback.print_exc()
            return _numpy_fallback(output, target)


def _kernel_device(output, target):
    global LAST_RESULTS
    from concourse import bass_utils
    from concourse.bass_interp import get_hw_module

    if "nc" not in _cache:
        _cache["nc"] = _build()
    nc = _cache["nc"]

    xy_img0, xy_big, xy_img3, xy_img4, xy_small = _pack(output, target)
    ones = _ones_blk()
    in_maps = [
        {
            "xy_img0": xy_img0[i],
            "xy_big": xy_big[i],
            "xy_img3": xy_img3[i],
            "xy_img4": xy_img4[i],
            "xy_small": xy_small[i],
            "ones_blk": ones,
        }
        for i in range(N_CORES)
    ]

    trace = bool(int(os.environ.get("BASSK_TRACE", "0")))
    tmpdir = None
    if trace:
        import tempfile

        _install_ntff_hook()
        tmpdir = tempfile.mkdtemp(prefix="bassk_trace_")
        global LAST_TRACE_DIR
        LAST_TRACE_DIR = tmpdir
    old_m = nc.m
    nc.m = get_hw_module(nc.m)
    try:
        results = bass_utils.run_bass_kernel_spmd(
            nc, in_maps, core_ids=list(range(N_CORES)), trace=trace, tmpdir=tmpdir
        )
    finally:
        nc.m = old_m
    LAST_RESULTS = results

    vals = np.stack([r["res"] for r in results.results])  # [8, 64, 32]
    scv = np.stack([r["res_sc"] for r in results.results])  # [8, 5, 64, 512]
    # res = patch-sum grid for imgs 4,5 chunk 3; res_sc slots: ps01, ps23,
    # sc0, sc1, sc2 (rows 0:32 even image / 32:64 odd image of each pair).
    # Finish the 16-col patch sums for the raw tiles on the host.
    s = scv.reshape(N_CORES, 5, 64, 32, 16).sum(axis=4, dtype=np.float32)
    i4 = np.maximum(
        s[:, 2:5, 0:32].max(axis=(1, 2, 3)), vals[:, 0:32].max(axis=(1, 2))
    )
    i5 = np.maximum(
        s[:, 2:5, 32:64].max(axis=(1, 2, 3)), vals[:, 32:64].max(axis=(1, 2))
    )
    mx = np.stack(
        [
            s[:, 0, 0:32].max(axis=(1, 2)),
            s[:, 0, 32:64].max(axis=(1, 2)),
            s[:, 1, 0:32].max(axis=(1, 2)),
            s[:, 1, 32:64].max(axis=(1, 2)),
            i4,
            i5,
        ],
        axis=1,
    )  # [8, 6]
    max_patch_loss = np.maximum(mx.astype(np.float32) / np.float32(P * P), 0.0)
    return np.float32(max_patch_loss.mean(dtype=np.float32))
